# revision 8
# baseline (speedup 1.0000x reference)
"""Trainium2 kernel for nn_Net_1_2_3 (hierarchical 1-2-3-GNN).

Split: the 8 NeuronCores run the heavy NNConv edge work (edge-MLP
relu(ea@W1+b1)@W2 producing per-edge [mi,mo] weight matrices, contracted
against gathered source features into per-edge messages) — ~56 GMAC that
dominates a single-CPU host. Edges are sharded 8 ways. One unified Bass
program (shapes padded to mi=mo=64) is compiled once and dispatched once
per layer; the host does the cheap index bookkeeping between dispatches
(scatter-add of messages, root matmul, ELU) and the small pooled levels.
"""
import sys
import time
import numpy as np

sys.path.insert(0, "/opt/trn_rl_repo")

N, E = 16384, 65536
N2, A2, E2 = 65536, 131072, 262144
N3, A3, E3 = 65536, 196608, 262144
B = 256
F_IN = 16
NCORES = 8
EC = E // NCORES  # 8192 edges per core
CHUNK = 2048
MIMO = [(16, 32), (32, 64), (64, 64)]

_CACHE = {}
_T0 = time.perf_counter()


def _tlog(msg):
    print(f"[kernel +{time.perf_counter()-_T0:7.2f}s] {msg}", file=sys.stderr)


def _build_msg_kernel():
    """Per-layer NNConv message kernel, unified padded shapes.

    Per core: eaT [8, EC] (7 edge attrs + zero row, transposed),
    w1p [8, 128], b1 [128, 1], w2p [128, 4096] (padded [128, i*64+o]),
    xsT [64, EC] (gathered source features, transposed, rows >= mi zero).
    Output msgT [64, EC]: msgT[o, e] = sum_i xsT[i, e] * We[e, i, o]
    with We = relu(eaT^T w1p + b1) @ w2p, computed as PSUM accumulation
    over i of matmuls w2p_i^T @ (hT * xsT[i]).
    """
    import concourse.bass as bass
    import concourse.bacc as bacc
    import concourse.tile as tile
    import concourse.mybir as mybir

    dt = mybir.dt
    nc = bacc.Bacc(None, target_bir_lowering=False, debug=False)

    eaT_ext = nc.dram_tensor("eaT", [8, EC], dt.float32, kind="ExternalInput")
    w1_ext = nc.dram_tensor("w1p", [8, 128], dt.float32, kind="ExternalInput")
    b1_ext = nc.dram_tensor("b1", [128, 1], dt.float32, kind="ExternalInput")
    w2_ext = nc.dram_tensor("w2p", [128, 4096], dt.float32, kind="ExternalInput")
    xs_ext = nc.dram_tensor("xsT", [64, EC], dt.float32, kind="ExternalInput")
    msg_ext = nc.dram_tensor("msgT", [64, EC], dt.float32, kind="ExternalOutput")

    NCH = EC // CHUNK

    with tile.TileContext(nc) as tc:
        with (
            tc.tile_pool(name="cst", bufs=1) as cst,
            tc.tile_pool(name="pool", bufs=3) as pool,
            tc.tile_pool(name="psA", bufs=2, space="PSUM") as psA,
            tc.tile_pool(name="psB", bufs=1, space="PSUM") as psB,
        ):
            eaT = cst.tile([8, EC], dt.float32)
            w1 = cst.tile([8, 128], dt.float32)
            b1 = cst.tile([128, 1], dt.float32)
            w2 = cst.tile([128, 4096], dt.float32)
            msgT = cst.tile([64, EC], dt.float32)
            hT = cst.tile([128, EC], dt.float32)
            nc.gpsimd.dma_start(eaT[:], eaT_ext[:])
            nc.gpsimd.dma_start(w1[:], w1_ext[:])
            nc.gpsimd.dma_start(b1[:], b1_ext[:])
            nc.gpsimd.dma_start(w2[:], w2_ext[:])

            # edge MLP: hT [128, EC] = relu(w1p^T @ eaT + b1)
            for c in range(EC // 512):
                hp = psA.tile([128, 512], dt.float32, tag="hp")
                nc.tensor.matmul(hp[:], w1[:], eaT[:, c * 512:(c + 1) * 512])
                nc.scalar.activation(
                    hT[:, c * 512:(c + 1) * 512], hp[:],
                    mybir.ActivationFunctionType.Relu, bias=b1[:], scale=1.0,
                )

            # msgT[o, e] = sum_i w2p[:, i*64+o]^T @ (hT[:, e] * xsT[i, e])
            # xsT row i is DMA-replicated across partitions, multiplied into
            # hT elementwise, and the 64 contributions accumulate in PSUM.
            for ch in range(NCH):
                lo = ch * CHUNK
                hi = lo + CHUNK
                mp = psB.tile([64, CHUNK], dt.float32, tag="mp")
                for i in range(64):
                    hxb = pool.tile([128, CHUNK], dt.float32, tag="hxb")
                    nc.gpsimd.dma_start(
                        hxb[:],
                        xs_ext[i:i + 1, lo:hi].to_broadcast([128, CHUNK]),
                    )
                    hxm = pool.tile([128, CHUNK], dt.float32, tag="hxm")
                    nc.vector.tensor_tensor(
                        hxm[:], hT[:, lo:hi], hxb[:],
                        op=mybir.AluOpType.mult,
                    )
                    for j in range(CHUNK // 512):
                        nc.tensor.matmul(
                            mp[:, j * 512:(j + 1) * 512],
                            w2[:, i * 64:(i + 1) * 64],
                            hxm[:, j * 512:(j + 1) * 512],
                            start=(i == 0), stop=(i == 63),
                        )
                nc.scalar.activation(
                    msgT[:, lo:hi], mp[:],
                    mybir.ActivationFunctionType.Copy, bias=0.0, scale=1.0,
                )
            nc.gpsimd.dma_start(msg_ext[:], msgT[:])
    nc.compile()
    return nc


def _get_nc():
    if "nc" not in _CACHE:
        _tlog("building device kernel")
        _CACHE["nc"] = _build_msg_kernel()
        _tlog("device kernel compiled")
    return _CACHE["nc"]


def _elu(v):
    return np.where(v > 0, v, np.expm1(np.minimum(v, 0.0)))


def _segsum(v, idx, n):
    out = np.zeros((n, v.shape[1]), np.float32)
    np.add.at(out, idx, v)
    return out


def _nnconv_layers_device(x, ei, ea, params):
    """Run the 3 NNConv layers; messages on device, glue on host."""
    from concourse.bass_utils import run_bass_kernel_spmd

    nc = _get_nc()
    src, dst = ei[0], ei[1]

    eaT_full = np.zeros((8, E), np.float32)
    eaT_full[:7] = ea.T
    eaT_cores = [np.ascontiguousarray(eaT_full[:, c * EC:(c + 1) * EC])
                 for c in range(NCORES)]

    h = x
    for li, (mi, mo) in enumerate(MIMO):
        W1, b1, W2, b2, root, bias = params[li]
        w1p = np.zeros((8, 128), np.float32)
        w1p[:7] = W1
        b1p = np.ascontiguousarray(b1.reshape(128, 1), dtype=np.float32)
        w2p = np.zeros((128, 64, 64), np.float32)
        w2p[:, :mi, :mo] = W2.reshape(128, mi, mo)
        w2p = w2p.reshape(128, 4096)

        hpadT = np.zeros((64, N), np.float32)
        hpadT[:mi] = h.T
        in_maps = []
        for c in range(NCORES):
            sl = slice(c * EC, (c + 1) * EC)
            xsT = np.ascontiguousarray(hpadT[:, src[sl]])
            in_maps.append({
                "eaT": eaT_cores[c], "w1p": w1p, "b1": b1p,
                "w2p": w2p, "xsT": xsT,
            })
        _tlog(f"layer {li}: dispatching")
        res = run_bass_kernel_spmd(nc, in_maps, core_ids=list(range(NCORES)))
        _tlog(f"layer {li}: dispatch done")
        msg = np.empty((E, mo), np.float32)
        for c in range(NCORES):
            sl = slice(c * EC, (c + 1) * EC)
            msg[sl] = res.results[c]["msgT"][:mo].T
        if np.any(b2):
            msg += h[src] @ b2.reshape(mi, mo)
        agg = _segsum(msg, dst, N)
        h = _elu(h @ root + agg + bias)
    return h


def _nnconv_layers_host(x, ei, ea, params):
    h = x
    for li, (mi, mo) in enumerate(MIMO):
        W1, b1, W2, b2, root, bias = params[li]
        hmlp = np.maximum(ea @ W1 + b1, 0.0) @ W2 + b2
        We = hmlp.reshape(-1, mi, mo)
        msg = np.matmul(h[ei[0]][:, None, :], We)[:, 0, :]
        agg = _segsum(msg, ei[1], N)
        h = _elu(h @ root + agg + bias)
    return h


def kernel(**inputs):
    inp = {k: np.asarray(v) for k, v in inputs.items()}
    x = inp["x"].astype(np.float32)
    ei = inp["edge_index"].astype(np.int64)
    ea = inp["edge_attr"].astype(np.float32)

    params = []
    for li in range(3):
        params.append((
            inp[f"nn{li+1}_W1"].astype(np.float32),
            inp[f"nn{li+1}_b1"].astype(np.float32),
            inp[f"nn{li+1}_W2"].astype(np.float32),
            inp[f"nn{li+1}_b2"].astype(np.float32),
            inp[f"conv{li+1}_root"].astype(np.float32),
            inp[f"conv{li+1}_bias"].astype(np.float32),
        ))

    _tlog("kernel() start")
    try:
        h = _nnconv_layers_device(x, ei, ea, params)
        _tlog("stage A done")
    except Exception:
        import traceback
        traceback.print_exc()
        print("kernel.py: device path failed, falling back to host",
              file=sys.stderr)
        h = _nnconv_layers_host(x, ei, ea, params)

    x_1 = _segsum(h, inp["batch"].astype(np.int64), B)

    def pool_level(node_idx, cluster_idx, iso, ei_l, batch_l, wrel1, wroot1,
                   bias1, wrel2, wroot2, bias2, ncl):
        s = _segsum(h[node_idx], cluster_idx, ncl)
        cnt = np.bincount(cluster_idx, minlength=ncl).astype(np.float32)
        hp = s / np.maximum(cnt, 1.0)[:, None]
        hc = np.concatenate([hp, iso], axis=1)
        src_l, dst_l = ei_l[0], ei_l[1]
        # project before gather/scatter: segsum(hc[src]) @ W == segsum((hc@W)[src])
        y = hc @ wrel1
        hc2 = _elu(_segsum(y[src_l], dst_l, ncl) + hc @ wroot1 + bias1)
        y2 = hc2 @ wrel2
        hc3 = _elu(_segsum(y2[src_l], dst_l, ncl) + hc2 @ wroot2 + bias2)
        return _segsum(hc3, batch_l, B)

    x_2 = pool_level(
        inp["assign2_node"].astype(np.int64),
        inp["assign2_cluster"].astype(np.int64),
        inp["iso_type_2"].astype(np.float32),
        inp["edge_index_2"].astype(np.int64),
        inp["batch_2"].astype(np.int64),
        inp["conv4_Wrel"].astype(np.float32),
        inp["conv4_Wroot"].astype(np.float32),
        inp["conv4_bias"].astype(np.float32),
        inp["conv5_Wrel"].astype(np.float32),
        inp["conv5_Wroot"].astype(np.float32),
        inp["conv5_bias"].astype(np.float32), N2)
    x_3 = pool_level(
        inp["assign3_node"].astype(np.int64),
        inp["assign3_cluster"].astype(np.int64),
        inp["iso_type_3"].astype(np.float32),
        inp["edge_index_3"].astype(np.int64),
        inp["batch_3"].astype(np.int64),
        inp["conv6_Wrel"].astype(np.float32),
        inp["conv6_Wroot"].astype(np.float32),
        inp["conv6_bias"].astype(np.float32),
        inp["conv7_Wrel"].astype(np.float32),
        inp["conv7_Wroot"].astype(np.float32),
        inp["conv7_bias"].astype(np.float32), N3)

    _tlog("stage B done")
    xc = np.concatenate([x_1, x_2, x_3], axis=1)
    xc = np.concatenate([xc, xc], axis=1)
    o = _elu(xc @ inp["fc1_W"].astype(np.float32) + inp["fc1_b"])
    o = _elu(o @ inp["fc2_W"].astype(np.float32) + inp["fc2_b"])
    o = o @ inp["fc3_W"].astype(np.float32) + inp["fc3_b"]
    return o.reshape(-1).astype(np.float32)
